# revision 31
# baseline (speedup 1.0000x reference)
"""BitTransformerBlock on 8 Trainium2 NeuronCores.

Token-parallel sharding: the flattened (B*S)=4096 tokens are split 512 per
core; cores 0-3 hold batch 0, cores 4-7 batch 1.  Each core computes LN1 and
the q/k/v projections for its own tokens; in-kernel AllGathers (replica
groups [0..3], [4..7]) share K and V across each batch group in fp8, and
everything downstream (attention over the full 2048-token context, out-proj,
LN2, the quantized FFN) is token-local.

Structure:
  - K/V are cast to fp8e4m3 before the collectives, which are split into
    four half-gathers issued K1, V1, K2, V2 on the serial CC stream so the
    first-half scores and the first AV matmuls start as early as possible.
  - Per-head softmax-denominator ones-columns are interleaved into the V
    bounce rows on the SBUF side, so they ride through the collective and
    the unpack DMA is fully contiguous.
  - LN1/LN2 means and rstd come from bn_stats; the normalize-scale-shift is
    a single fused scalar-engine activation.  nx is transposed feature-major
    with PE transposes (no DRAM round trip).
  - Attention is exp(ACT)-paced: scores run 3 head-pairs ahead of the
    AV accumulation; exp tiles are written to fp8 and consumed by
    DoubleRow AV matmuls (contraction pairs of key chunks); junk-matmul
    "flip bursts" keep the PE HAM clock-gate at full rate through the
    ACT-paced stretches.
  - The FFN uses constant-scale activation quantization: hq = fp8(LN2*32)
    with the dequant s1/32 folded into the gelu activation scale, and the
    gelu output is written straight to fp8 (its rounding replaces the
    reference's per-token int8 grid; dequant s2 is constant).  mm1 is
    FF-major (lhsT = w1 pairs), mm2 token-major (lhsT = y1q pairs), both
    fp8 DoubleRow, so no activation matrix is ever transposed through DRAM;
    hq needs 32 PE-transposes.
"""

import numpy as np
import ml_dtypes

import concourse.bacc as bacc
import concourse.bass as bass
import concourse.mybir as mybir
import concourse.tile as tile
from concourse.bass_interp import get_hw_module
from concourse.bass_utils import run_bass_kernel_spmd
from concourse.masks import make_identity

F32 = mybir.dt.float32
BF16 = mybir.dt.bfloat16
FP8 = mybir.dt.float8e4
AF = mybir.ActivationFunctionType
OP = mybir.AluOpType
DR = mybir.MatmulPerfMode.DoubleRow
DRSW = mybir.MatmulPerfMode.DoubleRowSwInterleave

N_CORES = 8
B, S, D, H, FF = 2, 2048, 1024, 16, 4096
HD = D // H                 # 64
NTOK = B * S                # 4096
TOK = NTOK // N_CORES       # 512 tokens per core
TCH = TOK // 128            # 4 token chunks per core
DCH = D // 128              # 8
FFCH = FF // 128            # 32
NKC = S // 128              # 16 key chunks per batch
GROUPS = [[0, 1, 2, 3], [4, 5, 6, 7]]
CORES_PER_B = 4
EPS = 1e-5
MAGIC = 12582912.0          # 1.5 * 2**23: fp32 round-to-nearest-even trick
INV_SQRT_HD = 1.0 / 8.0
GELU_MIN = 0.17             # |min gelu| = 0.16995 (reference for error analysis)
QC = 32.0                   # constant act-quant scale: hq = fp8(LN2(x2)*QC)
SCH_A = 12102203.16 / 8.0   # Schraudolph exp(s/8): 2^23/ln2 scaled by 1/8
SCH_B = float((127 << 23) - 368000)


def _bcast_part(ap, parts):
    """View a [1, F] (or [F]) AP as [parts, F] via a zero-stride partition dim."""
    inner = [list(e) for e in ap.ap if e[1] != 1] or [[1, 1]]
    return bass.AP(tensor=ap.tensor, offset=ap.offset, ap=[[0, parts]] + inner)


def build_program(s1, s2, biases, sim_gelu=False):
    nc = bacc.Bacc("TRN2", target_bir_lowering=False, debug=False,
                   num_devices=N_CORES)

    x_in = nc.dram_tensor("x_sh", [TOK, D], BF16, kind="ExternalInput")
    wq_in = nc.dram_tensor("wqT", [D, D], BF16, kind="ExternalInput")
    wk_in = nc.dram_tensor("wkT", [D, D], BF16, kind="ExternalInput")
    wv_in = nc.dram_tensor("wvT", [D, D], BF16, kind="ExternalInput")
    wo_in = nc.dram_tensor("woT", [D, D], BF16, kind="ExternalInput")
    w1_in = nc.dram_tensor("w1T", [D, FF], FP8, kind="ExternalInput")
    w2_in = nc.dram_tensor("w2T", [128, (FFCH // 2) * D * 2], FP8, kind="ExternalInput")
    ident_in = nc.dram_tensor("ident", [128, 128], BF16, kind="ExternalInput")
    out_d = nc.dram_tensor("out", [TOK, D], F32, kind="ExternalOutput")

    ext = {}
    if biases["ln1_g"]:
        ext["ln1_g"] = nc.dram_tensor("ln1_g", [D], F32, kind="ExternalInput")
    if biases["ln1_b"]:
        ext["ln1_b"] = nc.dram_tensor("ln1_b", [D], F32, kind="ExternalInput")
    if biases["ln2_g"]:
        ext["ln2_g"] = nc.dram_tensor("ln2_g", [D], F32, kind="ExternalInput")
    if biases["ln2_b"]:
        ext["ln2_b"] = nc.dram_tensor("ln2_b", [D], F32, kind="ExternalInput")
    if biases["in_proj_b"]:
        ext["in_b"] = nc.dram_tensor("in_b", [3 * D], F32, kind="ExternalInput")
    if biases["out_proj_b"]:
        ext["out_b"] = nc.dram_tensor("out_b", [D], F32, kind="ExternalInput")
    if biases["b1"]:
        ext["b1"] = nc.dram_tensor("b1", [FF], F32, kind="ExternalInput")
    if biases["b2"]:
        ext["b2"] = nc.dram_tensor("b2", [D], F32, kind="ExternalInput")

    with tile.TileContext(nc) as tc:
        _emit(nc, tc, x_in, wq_in, wk_in, wv_in, wo_in, w1_in, w2_in,
              ident_in, out_d, ext, s1, s2, biases, sim_gelu)
    nc.compile()
    return nc


def _scope(nc, name):
    sid = nc.enter_named_scope(name, False)
    return (name, sid[0] if isinstance(sid, tuple) else sid)


def _unscope(nc, tok):
    nc.leave_named_scope(tok[0], tok[1], False)


def _emit(nc, tc, x_in, wq_in, wk_in, wv_in, wo_in, w1_in, w2_in, ident_in,
          out_d, ext, s1, s2, biases, sim_gelu=False):
    gelu_func = AF.Tanh if sim_gelu else AF.Gelu
    from contextlib import ExitStack

    es_top = ExitStack()
    dram = es_top.enter_context(tc.tile_pool(name="dram", bufs=1, space="DRAM"))
    const = es_top.enter_context(tc.tile_pool(name="const", bufs=1))
    stats = es_top.enter_context(tc.tile_pool(name="stats", bufs=4))

    VA = H * (HD + 1)           # 1040: V row with a ones column per head
    kT_bounce = dram.tile([D, TOK], FP8)
    v_bounce1 = dram.tile([TOK, VA // 2], FP8)
    v_bounce2 = dram.tile([TOK, VA // 2], FP8)
    kT_all1 = dram.tile([CORES_PER_B * (D // 2), TOK], FP8)
    kT_all2 = dram.tile([CORES_PER_B * (D // 2), TOK], FP8)
    v_all1 = dram.tile([S, VA // 2], FP8)
    v_all2 = dram.tile([S, VA // 2], FP8)
    den_dram = dram.tile([H, TOK], F32)
    dq1_dram = dram.tile([TOK], F32)
    a2_dram = dram.tile([TOK], F32)

    eps_t = const.tile([128, 1], F32)
    nc.vector.memset(eps_t[:], EPS)
    magic_t = const.tile([128, 1], F32)
    nc.vector.memset(magic_t[:], MAGIC)
    ident = const.tile([128, 128], BF16, tag="ident")
    nc.sync.dma_start(out=ident[:], in_=ident_in[:])

    def load_bcast(name, width, src_ap):
        t = const.tile([128, width], F32, tag=f"bc_{name}")
        nc.sync.dma_start(out=t[:], in_=_bcast_part(src_ap, 128))
        return t

    g1_bc = load_bcast("g1", D, ext["ln1_g"][:]) if biases["ln1_g"] else None
    b1ln_bc = load_bcast("b1ln", D, ext["ln1_b"][:]) if biases["ln1_b"] else None
    g2_bc = load_bcast("g2", D, ext["ln2_g"][:]) if biases["ln2_g"] else None
    b2ln_bc = load_bcast("b2ln", D, ext["ln2_b"][:]) if biases["ln2_b"] else None
    bv_bc = (load_bcast("bv", D, ext["in_b"][2 * D:3 * D])
             if biases["in_proj_b"] else None)
    bo_bc = load_bcast("bo", D, ext["out_b"][:]) if biases["out_proj_b"] else None
    bf2_bc = load_bcast("bf2", D, ext["b2"][:]) if biases["b2"] else None
    if biases["in_proj_b"]:
        bq_fm = const.tile([128, DCH], F32, tag="bq_fm")
        nc.sync.dma_start(out=bq_fm[:], in_=ext["in_b"][0:D].rearrange("(c p) -> p c", p=128))
        bk_fm = const.tile([128, DCH], F32, tag="bk_fm")
        nc.sync.dma_start(out=bk_fm[:], in_=ext["in_b"][D:2 * D].rearrange("(c p) -> p c", p=128))
    if biases["b1"]:
        b1_fm = const.tile([128, FFCH], F32, tag="b1_fm")
        nc.sync.dma_start(out=b1_fm[:], in_=ext["b1"][:].rearrange("(c p) -> p c", p=128))

    # ---- long-lived pools (stack allocator: open early, close late) -----
    es_D = ExitStack()
    pD = es_D.enter_context(tc.tile_pool(name="pD", bufs=1))
    x2 = pD.tile([128, TCH, D], F32, tag="x2")
    s2t = pD.tile([128, 1], F32, tag="s2t")
    nc.vector.memset(s2t[:], float(s2))

    es_W1 = ExitStack()
    pW1 = es_W1.enter_context(tc.tile_pool(name="pW1", bufs=1))
    w1_sb = pW1.tile([128, DCH, FF], FP8, tag="w1")  # 32KB/part

    es_A = ExitStack()
    pA = es_A.enter_context(tc.tile_pool(name="pA", bufs=1))
    x_bf = pA.tile([128, TCH, D], BF16, tag="x")

    es_C = ExitStack()
    pC = es_C.enter_context(tc.tile_pool(name="pC", bufs=1))
    oT = pC.tile([128, DCH, TOK], BF16, tag="oT")

    es_B = ExitStack()
    pB = es_B.enter_context(tc.tile_pool(name="pB", bufs=1))
    qT = pB.tile([128, DCH, TOK], BF16, tag="qT")
    KT = pB.tile([128, DCH, CORES_PER_B, 512], FP8, tag="KT")
    Vaug = pB.tile([128, NKC, H * (HD + 1)], FP8, tag="Va")

    # ---- stage 0: load x (bf16) -----------------------------------------
    x_v = x_in.rearrange("(i p) d -> p i d", p=128)
    for i in range(TCH):
        nc.sync.dma_start(out=x_bf[:, i, :], in_=x_v[:, i, :])

    # PE warm-up: keep the HAM clock-gate at 8/8 through the prologue so the
    # first real matmuls run at full clock.  Junk matmuls on the identity.
    tok_wm = _scope(nc, "warm")
    es_wm = ExitStack()
    ps_wm = es_wm.enter_context(tc.tile_pool(name="ps_wm", bufs=1, space="PSUM"))
    wm = ps_wm.tile([128, 128], F32, tag="wm")
    for _ in range(64):
        nc.tensor.matmul(wm[:], lhsT=ident[:], rhs=ident[:], start=True, stop=True)
    es_wm.close()
    _unscope(nc, tok_wm)

    # ---- stage 1: LN1 -> nx (bf16, token-major) -> DRAM -----------------
    def layer_norm_chunk(src_ap, g_bc, b_bc, out_tile, post_scale=1.0):
        """out = (src - m) * rstd * post_scale [* g] [+ b] via one fused ACT op."""
        st = stats.tile([128, 2, 6], F32, tag="bnst")
        nc.vector.bn_stats(out=st[:, 0, :], in_=src_ap[:, 0:512])
        nc.vector.bn_stats(out=st[:, 1, :], in_=src_ap[:, 512:1024])
        mv = stats.tile([128, 2], F32, tag="mv")
        nc.vector.bn_aggr(out=mv[:], in_=st[:])
        r = stats.tile([128, 1], F32, tag="rstd")
        nc.scalar.activation(out=r[:], in_=mv[:, 1:2], func=AF.Ln, bias=eps_t[:])
        nc.scalar.activation(out=r[:], in_=r[:], func=AF.Exp, scale=-0.5)
        fold = post_scale != 1.0 and g_bc is None and b_bc is None
        if fold:
            nc.vector.tensor_scalar_mul(out=r[:], in0=r[:], scalar1=float(post_scale))
        nmr = stats.tile([128, 1], F32, tag="nmr")
        nc.vector.tensor_tensor(out=nmr[:], in0=mv[:, 0:1], in1=r[:], op=OP.mult)
        nc.vector.tensor_scalar_mul(out=nmr[:], in0=nmr[:], scalar1=-1.0)
        nc.scalar.activation(out=out_tile, in_=src_ap, func=AF.Identity,
                             scale=r[:], bias=nmr[:])
        if g_bc is not None:
            nc.vector.tensor_mul(out=out_tile, in0=out_tile, in1=g_bc[:])
        if b_bc is not None:
            nc.vector.tensor_add(out=out_tile, in0=out_tile, in1=b_bc[:])
        if post_scale != 1.0 and not fold:
            nc.vector.tensor_scalar_mul(out=out_tile, in0=out_tile,
                                        scalar1=float(post_scale))
        return mv, r

    # weight loads first so they don't queue behind LN1-dependent DMAs
    es_X = ExitStack()
    pX = es_X.enter_context(tc.tile_pool(name="pX", bufs=1))
    pW = es_X.enter_context(tc.tile_pool(name="pWqkv", bufs=1))
    wk_sb = pW.tile([128, DCH, D], BF16, tag="wk")
    nc.sync.dma_start(out=wk_sb[:], in_=wk_in.rearrange("(c p) f -> p c f", p=128))
    wv_sb = pW.tile([128, DCH, D], BF16, tag="wv")
    nc.sync.dma_start(out=wv_sb[:], in_=wv_in.rearrange("(c p) f -> p c f", p=128))
    wq_sb = pW.tile([128, DCH, D], BF16, tag="wq")
    nc.sync.dma_start(out=wq_sb[:], in_=wq_in.rearrange("(c p) f -> p c f", p=128))

    tok_ln1 = _scope(nc, "ln1")
    nxT = pX.tile([128, DCH, TOK], BF16, tag="nxT")
    es_1 = ExitStack()
    s1p = es_1.enter_context(tc.tile_pool(name="s1p", bufs=3))
    ps1 = es_1.enter_context(tc.tile_pool(name="ps1", bufs=4, space="PSUM"))
    for i in range(TCH):
        nxt = s1p.tile([128, D], BF16, tag="nx")
        layer_norm_chunk(x_bf[:, i, :], g1_bc, b1ln_bc, nxt[:])
        for dc in range(DCH):
            pst = ps1.tile([128, 128], BF16, tag="ps1t")
            nc.tensor.transpose(pst[:], nxt[:, dc * 128:(dc + 1) * 128], ident[:])
            nc.vector.tensor_copy(out=nxT[:, dc, i * 128:(i + 1) * 128], in_=pst[:])
    es_1.close()
    _unscope(nc, tok_ln1)

    tok_inp = _scope(nc, "inproj")
    es_3 = ExitStack()
    ps3 = es_3.enter_context(tc.tile_pool(name="ps3", bufs=4, space="PSUM"))
    s3 = es_3.enter_context(tc.tile_pool(name="s3", bufs=3))

    # k projection, feature-major, cast fp8 -> bounce; the K AllGather is
    # split in two so the first half (heads 0-7) lands much earlier
    def k_half(half, chunked=False):
        for fo in range(half * 4, half * 4 + 4):
            ps = ps3.tile([128, 512], F32, tag="ps")
            if chunked:
                # token-chunked so the first chunks only need the first LN1
                # outputs -> the K bounce (and its AllGather) starts earliest
                for tch in range(TCH):
                    sl = slice(tch * 128, (tch + 1) * 128)
                    for dc in range(DCH):
                        nc.tensor.matmul(ps[:, sl],
                                         lhsT=wk_sb[:, dc, fo * 128:(fo + 1) * 128],
                                         rhs=nxT[:, dc, sl],
                                         start=(dc == 0), stop=(dc == DCH - 1))
            else:
                for dc in range(DCH):
                    nc.tensor.matmul(ps[:], lhsT=wk_sb[:, dc, fo * 128:(fo + 1) * 128],
                                     rhs=nxT[:, dc, :], start=(dc == 0),
                                     stop=(dc == DCH - 1))
            kc_sb = s3.tile([128, 512], FP8, tag="kcp")
            if biases["in_proj_b"]:
                nc.scalar.activation(out=kc_sb[:], in_=ps[:], func=AF.Identity,
                                     bias=bk_fm[:, fo:fo + 1])
            else:
                nc.vector.tensor_copy(out=kc_sb[:], in_=ps[:])
            nc.sync.dma_start(
                out=kT_bounce.rearrange("(c p) t -> p c t", p=128)[:, fo, :],
                in_=kc_sb[:])

    k_half(0, chunked=True)
    tok_cK = _scope(nc, "collK")
    nc.gpsimd.collective_compute(
        "AllGather", OP.bypass, replica_groups=GROUPS,
        ins=[kT_bounce[0:D // 2, :].opt()], outs=[kT_all1.opt()])
    _unscope(nc, tok_cK)
    _unscope(nc, tok_inp)

    tok_vp = _scope(nc, "vproj")
    # v projection, token-major, cast fp8, interleave per-head ones columns
    # (the ones ride through the collective so the unpack DMA is contiguous)
    for to in range(TCH):
        vci = s3.tile([128, H, HD + 1], FP8, tag="vci", name=f"vci{to}")
        nc.vector.memset(vci[:], 1.0)
        pv = [ps3.tile([128, 512], F32, tag="ps", name=f"psv{to}_{i}")
              for i in range(2)]
        for dc in range(DCH):
            for f2 in range(2):
                nc.tensor.matmul(pv[f2][:], lhsT=nxT[:, dc, to * 128:(to + 1) * 128],
                                 rhs=wv_sb[:, dc, f2 * 512:(f2 + 1) * 512],
                                 start=(dc == 0), stop=(dc == DCH - 1))
        for f2 in range(2):
            dst = vci[:, f2 * 8:(f2 + 1) * 8, 0:HD]
            if biases["in_proj_b"]:
                nc.vector.tensor_add(out=dst, in0=pv[f2][:],
                                     in1=bv_bc[:, f2 * 512:(f2 + 1) * 512])
            else:
                nc.vector.tensor_copy(out=dst, in_=pv[f2][:])
        nc.sync.dma_start(
            out=v_bounce1.rearrange("(i p) f -> p i f", p=128)[:, to, :],
            in_=vci[:, 0:8, :])
        nc.sync.dma_start(
            out=v_bounce2.rearrange("(i p) f -> p i f", p=128)[:, to, :],
            in_=vci[:, 8:16, :])
    _unscope(nc, tok_vp)

    tok_cV = _scope(nc, "collV")
    nc.gpsimd.collective_compute(
        "AllGather", OP.bypass, replica_groups=GROUPS,
        ins=[v_bounce1.opt()], outs=[v_all1.opt()])
    _unscope(nc, tok_cV)

    tok_qp = _scope(nc, "qproj")
    for fo in range(DCH):
        ps = ps3.tile([128, 512], F32, tag="ps")
        for dc in range(DCH):
            nc.tensor.matmul(ps[:], lhsT=wq_sb[:, dc, fo * 128:(fo + 1) * 128],
                             rhs=nxT[:, dc, :], start=(dc == 0), stop=(dc == DCH - 1))
        if biases["in_proj_b"]:
            nc.scalar.activation(out=qT[:, fo, :], in_=ps[:], func=AF.Identity,
                                 bias=bq_fm[:, fo:fo + 1])
        else:
            nc.scalar.activation(out=qT[:, fo, :], in_=ps[:], func=AF.Copy)
    _unscope(nc, tok_qp)

    tok_k2 = _scope(nc, "kproj2")
    k_half(1)
    _unscope(nc, tok_k2)
    tok_cK2 = _scope(nc, "collK2")
    nc.gpsimd.collective_compute(
        "AllGather", OP.bypass, replica_groups=GROUPS,
        ins=[kT_bounce[D // 2:D, :].opt()], outs=[kT_all2.opt()])
    _unscope(nc, tok_cK2)
    tok_cV2 = _scope(nc, "collV2")
    nc.gpsimd.collective_compute(
        "AllGather", OP.bypass, replica_groups=GROUPS,
        ins=[v_bounce2.opt()], outs=[v_all2.opt()])
    _unscope(nc, tok_cV2)

    # prefetch FFN + out_proj weights (overlaps collectives/attention)
    nc.sync.dma_start(out=w1_sb[:], in_=w1_in.rearrange("(c p) f -> p c f", p=128))
    wo_sb = pC.tile([128, DCH, D], BF16, tag="wo")
    nc.sync.dma_start(out=wo_sb[:], in_=wo_in.rearrange("(c p) f -> p c f", p=128))

    tok_un = _scope(nc, "unpack")
    # unpack gathered K^T / V(+ones) into SBUF (fp8, all contiguous runs)
    HD2 = D // 2
    for c in range(CORES_PER_B):
        nc.sync.dma_start(
            out=KT[:, 0:4, c, :],
            in_=kT_all1[c * HD2:(c + 1) * HD2, :].rearrange("(dch p) t -> p dch t", p=128))
    for c in range(CORES_PER_B):
        nc.sync.dma_start(
            out=KT[:, 4:8, c, :],
            in_=kT_all2[c * HD2:(c + 1) * HD2, :].rearrange("(dch p) t -> p dch t", p=128))
    va1 = v_all1.rearrange("(kc p) f -> p kc f", p=128)
    nc.sync.dma_start(out=Vaug[:, :, 0:VA // 2], in_=va1[:])
    va2 = v_all2.rearrange("(kc p) f -> p kc f", p=128)
    nc.sync.dma_start(out=Vaug[:, :, VA // 2:VA], in_=va2[:])
    es_3.close()
    es_X.close()
    _unscope(nc, tok_un)

    tok_at = _scope(nc, "attn")
    # ---- attention: ACT(exp)-paced pipeline, AV lags scores by 3 hp ------
    es_5 = ExitStack()
    ps_s = es_5.enter_context(tc.tile_pool(name="ps_s", bufs=3, space="PSUM"))
    ps_av = es_5.enter_context(tc.tile_pool(name="ps_av", bufs=2, space="PSUM"))
    s5e = es_5.enter_context(tc.tile_pool(name="s5e", bufs=60))
    s5d = es_5.enter_context(tc.tile_pool(name="s5d", bufs=3))

    exp_tiles = {}

    def emit_scores(hp):
        for g in range(NKC // 2):
            pss = [ps_s.tile([128, 2, 512], F32, tag="pss", name=f"pss{hp}_{g}_{i}")
                   for i in range(2)]
            for j in range(2):
                kc = 2 * g + j
                c, tcc = divmod(kc, 4)
                ksl = KT[:, hp, c, tcc * 128:(tcc + 1) * 128]
                nc.tensor.matmul(pss[0][:, j, :], lhsT=ksl[0:64, :],
                                 rhs=qT[0:64, hp, :], start=True, stop=True,
                                 tile_position=(0, 0))
                nc.tensor.matmul(pss[1][:, j, :], lhsT=ksl[64:128, :],
                                 rhs=qT[64:128, hp, :], start=True, stop=True,
                                 tile_position=(64, 0))
            for jh in range(2):
                e = s5e.tile([128, 512, 2], FP8, tag="exp", name=f"e{hp}_{g}_{jh}")
                nc.scalar.activation(out=e[:].rearrange("p t j -> p j t"),
                                     in_=pss[jh][:], func=AF.Exp,
                                     scale=INV_SQRT_HD)
                exp_tiles[(hp, jh, g)] = e

    def emit_av(hp):
        for jh in range(2):
            h = 2 * hp + jh
            pav = ps_av.tile([128, 512], F32, tag="pav", name=f"pav{hp}_{jh}")
            for g in range(NKC // 2):
                nc.tensor.matmul(pav[0:65, :],
                                 lhsT=Vaug[:, 2 * g:2 * g + 2, h * 65:h * 65 + 65],
                                 rhs=exp_tiles.pop((hp, jh, g))[:].rearrange("p t j -> p j t"),
                                 start=(g == 0), stop=(g == NKC // 2 - 1),
                                 perf_mode=DR)
            oun = s5d.tile([65, 512], F32, tag="oun", name=f"oun{hp}_{jh}")
            nc.vector.tensor_copy(out=oun[:], in_=pav[0:65, :])
            nc.sync.dma_start(out=den_dram[h:h + 1, :], in_=oun[64:65, :])
            dbc = s5d.tile([64, 512], F32, tag="dbc", name=f"dbc{hp}_{jh}")
            nc.sync.dma_start(out=dbc[:], in_=_bcast_part(den_dram[h:h + 1, :], 64))
            rdb = s5d.tile([64, 512], F32, tag="rdb", name=f"rdb{hp}_{jh}")
            nc.vector.reciprocal(out=rdb[:], in_=dbc[:])
            nc.vector.tensor_mul(out=oT[jh * 64:jh * 64 + 64, hp, :],
                                 in0=oun[0:64, :], in1=rdb[:])

    def emit_flip(hp):
        # one contiguous PE burst to flip the HAM clock-gate back to 8/8 if a
        # stall mid-attention dropped it to 4/8
        jp = ps_av.tile([128, 512], F32, tag="pav", name=f"flip{hp}")
        for i in range(56):
            nc.tensor.matmul(jp[:, 0:128], lhsT=ident[:], rhs=ident[:],
                             start=True, stop=True, skip_group_check=True)

    PIPE = 3
    for hp in range(PIPE):
        emit_scores(hp)
    for hp in range(H // 2):
        emit_av(hp)
        if hp in (1, 4, 6):
            emit_flip(hp)
        if hp + PIPE < H // 2:
            emit_scores(hp + PIPE)
    es_5.close()
    es_B.close()
    _unscope(nc, tok_at)

    tok_op = _scope(nc, "outproj")
    # ---- out_proj + residual (token-major) ------------------------------
    es_6 = ExitStack()
    ps6 = es_6.enter_context(tc.tile_pool(name="ps6", bufs=4, space="PSUM"))
    for to in range(TCH):
        po = [ps6.tile([128, 512], F32, tag="ps6", name=f"pso{to}_{i}")
              for i in range(2)]
        for dc in range(DCH):
            for f2 in range(2):
                nc.tensor.matmul(po[f2][:], lhsT=oT[:, dc, to * 128:(to + 1) * 128],
                                 rhs=wo_sb[:, dc, f2 * 512:(f2 + 1) * 512],
                                 start=(dc == 0), stop=(dc == DCH - 1))
        for f2 in range(2):
            dst = x2[:, to, f2 * 512:(f2 + 1) * 512]
            nc.vector.tensor_add(out=dst, in0=po[f2][:],
                                 in1=x_bf[:, to, f2 * 512:(f2 + 1) * 512])
            if biases["out_proj_b"]:
                nc.vector.tensor_add(out=dst, in0=dst,
                                     in1=bo_bc[:, f2 * 512:(f2 + 1) * 512])
    es_6.close()
    es_C.close()
    es_A.close()
    _unscope(nc, tok_op)

    tok_l2 = _scope(nc, "ln2q")
    # ---- LN2 + constant-scale act_quant -> hqT (fp8, feature-major) -----
    # hq = fp8(clamp(LN2(x2) * QC)); the dequant (s1/QC) folds into the gelu
    # scale, so no per-token amax/scale machinery is needed.
    es_E = ExitStack()
    pE = es_E.enter_context(tc.tile_pool(name="pE", bufs=1))
    hqT = pE.tile([128, DCH // 2, TOK, 2], FP8, tag="hqT")
    y1qT = pE.tile([128, FFCH, TOK], FP8, tag="y1qT")

    es_7 = ExitStack()
    s7 = es_7.enter_context(tc.tile_pool(name="s7", bufs=3))
    ps7 = es_7.enter_context(tc.tile_pool(name="ps7", bufs=4, space="PSUM"))

    for to in range(TCH):
        hs = s7.tile([128, D], BF16, tag="hs")
        layer_norm_chunk(x2[:, to, :], g2_bc, b2ln_bc, hs[:], post_scale=QC)
        hqi = s7.tile([128, D], BF16, tag="hq")
        nc.vector.tensor_scalar(out=hqi[:], in0=hs[:], scalar1=239.0,
                                scalar2=-239.0, op0=OP.min, op1=OP.max)
        for dc in range(DCH):
            pst = ps7.tile([128, 128], BF16, tag="pst")
            nc.tensor.transpose(pst[:], hqi[:, dc * 128:(dc + 1) * 128], ident[:])
            nc.vector.tensor_copy(
                out=hqT[:, dc // 2, to * 128:(to + 1) * 128, dc % 2],
                in_=pst[:])
    es_7.close()
    _unscope(nc, tok_l2)

    tok_f1 = _scope(nc, "ffn1")
    # ---- FFN mm1 (fp8 DoubleRow, FF-major) + gelu + running max ----------
    es_8 = ExitStack()
    ps8 = es_8.enter_context(tc.tile_pool(name="ps8", bufs=4, space="PSUM"))
    s8 = es_8.enter_context(tc.tile_pool(name="s8", bufs=3))
    es_W2 = ExitStack()
    pW2 = es_W2.enter_context(tc.tile_pool(name="pW2", bufs=1))
    w2_sb = pW2.tile([128, FFCH // 2, D, 2], FP8, tag="w2")
    nc.sync.dma_start(out=w2_sb[:], in_=w2_in[:])

    for ffo in range(FFCH):
        ps = ps8.tile([128, 512], F32, tag="ps8")
        for dcp in range(DCH // 2):
            nc.tensor.matmul(ps[:],
                             lhsT=w1_sb[:, 2 * dcp:2 * dcp + 2, ffo * 128:(ffo + 1) * 128],
                             rhs=hqT[:, dcp, :, :].rearrange("p t j -> p j t"),
                             start=(dcp == 0), stop=(dcp == DCH // 2 - 1),
                             perf_mode=DR)
        # y1 = ps * (s1/QC); gelu output goes straight to fp8 (its rounding
        # replaces the reference's int8 act_quant grid)
        if biases["b1"]:
            nc.scalar.activation(out=y1qT[:, ffo, :], in_=ps[:], func=gelu_func,
                                 scale=float(s1) / QC, bias=b1_fm[:, ffo:ffo + 1])
        else:
            nc.scalar.activation(out=y1qT[:, ffo, :], in_=ps[:], func=gelu_func,
                                 scale=float(s1) / QC)
    _unscope(nc, tok_f1)

    tok_f2 = _scope(nc, "ffn2")
    # ---- FFN mm2 (fp8 DoubleRow, token-major) + residual -> out ----------
    out_v = out_d.rearrange("(i p) d -> p i d", p=128)
    es_9 = ExitStack()
    ps9 = es_9.enter_context(tc.tile_pool(name="ps9", bufs=4, space="PSUM"))
    s9 = es_9.enter_context(tc.tile_pool(name="s9", bufs=3))
    for to in range(TCH):
        p9 = [ps9.tile([128, 512], F32, tag="ps9", name=f"ps9_{to}_{i}")
              for i in range(2)]
        for fcp in range(FFCH // 2):
            for f2 in range(2):
                nc.tensor.matmul(p9[f2][:],
                                 lhsT=y1qT[:, 2 * fcp:2 * fcp + 2, to * 128:(to + 1) * 128],
                                 rhs=w2_sb[:, fcp, f2 * 512:(f2 + 1) * 512, :].rearrange("p t j -> p j t"),
                                 start=(fcp == 0), stop=(fcp == FFCH // 2 - 1),
                                 perf_mode=DR)
        for f2 in range(2):
            outt = s9.tile([128, 512], F32, tag="outt")
            nc.vector.scalar_tensor_tensor(
                out=outt[:], in0=p9[f2][:], scalar=s2t[:],
                in1=x2[:, to, f2 * 512:(f2 + 1) * 512], op0=OP.mult, op1=OP.add)
            if biases["b2"]:
                nc.vector.tensor_add(out=outt[:], in0=outt[:],
                                     in1=bf2_bc[:, f2 * 512:(f2 + 1) * 512])
            nc.sync.dma_start(out=out_v[:, to, f2 * 512:(f2 + 1) * 512],
                              in_=outt[:])
    _unscope(nc, tok_f2)
    es_9.close()
    es_W2.close()
    es_8.close()
    es_E.close()
    es_W1.close()
    es_D.close()
    es_top.close()


_CACHE = {}


def _prepare(inputs):
    bf = ml_dtypes.bfloat16
    f8 = ml_dtypes.float8_e4m3
    x = np.ascontiguousarray(np.asarray(inputs["x"], dtype=np.float32))
    in_w = np.asarray(inputs["in_proj_w"], dtype=np.float32)
    out_w = np.asarray(inputs["out_proj_w"], dtype=np.float32)
    w1 = np.asarray(inputs["w1"], dtype=np.float32)
    w2 = np.asarray(inputs["w2"], dtype=np.float32)

    s1 = float(max(np.mean(np.abs(w1), dtype=np.float32), EPS))
    s2 = float(max(np.mean(np.abs(w2), dtype=np.float32), EPS))
    t1 = np.clip(np.round(w1 / np.float32(s1)), -1.0, 1.0).astype(np.float32)
    t2 = np.clip(np.round(w2 / np.float32(s2)), -1.0, 1.0).astype(np.float32)

    host = {
        "wqT": np.ascontiguousarray(in_w[0:D].T).astype(bf),
        "wkT": np.ascontiguousarray(in_w[D:2 * D].T).astype(bf),
        "wvT": np.ascontiguousarray(in_w[2 * D:3 * D].T).astype(bf),
        "woT": np.ascontiguousarray(out_w.T).astype(bf),
        "w1T": np.ascontiguousarray(t1.T).astype(f8),
        # w2 moving operand, pair-interleaved: [128, FFCH//2, D, 2]
        "w2T": np.ascontiguousarray(
            t2.T.reshape(16, 2, 128, D).transpose(2, 0, 3, 1)
            .reshape(128, -1)).astype(f8),
        "ident": np.eye(128, dtype=bf),
    }

    def nz(a):
        return bool(np.any(np.asarray(a) != 0.0))

    biases = {
        "ln1_g": bool(np.any(np.asarray(inputs["ln1_g"]) != 1.0)),
        "ln1_b": nz(inputs["ln1_b"]),
        "ln2_g": bool(np.any(np.asarray(inputs["ln2_g"]) != 1.0)),
        "ln2_b": nz(inputs["ln2_b"]),
        "in_proj_b": nz(inputs["in_proj_b"]),
        "out_proj_b": nz(inputs["out_proj_b"]),
        "b1": nz(inputs["b1"]),
        "b2": nz(inputs["b2"]),
    }
    extra = {}
    if biases["ln1_g"]:
        extra["ln1_g"] = np.asarray(inputs["ln1_g"], np.float32)
    if biases["ln1_b"]:
        extra["ln1_b"] = np.asarray(inputs["ln1_b"], np.float32)
    if biases["ln2_g"]:
        extra["ln2_g"] = np.asarray(inputs["ln2_g"], np.float32)
    if biases["ln2_b"]:
        extra["ln2_b"] = np.asarray(inputs["ln2_b"], np.float32)
    if biases["in_proj_b"]:
        extra["in_b"] = np.asarray(inputs["in_proj_b"], np.float32)
    if biases["out_proj_b"]:
        extra["out_b"] = np.asarray(inputs["out_proj_b"], np.float32)
    if biases["b1"]:
        extra["b1"] = np.asarray(inputs["b1"], np.float32)
    if biases["b2"]:
        extra["b2"] = np.asarray(inputs["b2"], np.float32)

    x_flat = x.reshape(NTOK, D).astype(bf)
    in_maps = []
    for c in range(N_CORES):
        m = {"x_sh": np.ascontiguousarray(x_flat[c * TOK:(c + 1) * TOK])}
        m.update(host)
        m.update(extra)
        in_maps.append(m)
    return in_maps, s1, s2, biases


def get_program(s1, s2, biases, for_hw=True, sim_gelu=False):
    key = (round(s1, 12), round(s2, 12), tuple(sorted(biases.items())), for_hw,
           sim_gelu)
    if key not in _CACHE:
        nc = build_program(s1, s2, biases, sim_gelu=sim_gelu)
        if for_hw:
            nc.m = get_hw_module(nc.m)
        _CACHE[key] = nc
    return _CACHE[key]


def kernel(**inputs):
    in_maps, s1, s2, biases = _prepare(inputs)
    nc = get_program(s1, s2, biases, for_hw=True)
    res = run_bass_kernel_spmd(nc, in_maps, list(range(N_CORES)))
    out = np.concatenate([res.results[c]["out"] for c in range(N_CORES)], axis=0)
    return out.reshape(B, S, D).astype(np.float32)


# revision 32
# speedup vs baseline: 8847.4392x; 8847.4392x over previous
"""BitTransformerBlock on 8 Trainium2 NeuronCores.

Token-parallel sharding: the flattened (B*S)=4096 tokens are split 512 per
core; cores 0-3 hold batch 0, cores 4-7 batch 1.  Each core computes LN1 and
the q/k/v projections for its own tokens; in-kernel AllGathers (replica
groups [0..3], [4..7]) share K and V across each batch group in fp8, and
everything downstream (attention over the full 2048-token context, out-proj,
LN2, the quantized FFN) is token-local.

Structure:
  - K/V are cast to fp8e4m3 before the collectives, which are split into
    four half-gathers issued K1, V1, K2, V2 on the serial CC stream so the
    first-half scores and the first AV matmuls start as early as possible.
  - Per-head softmax-denominator ones-columns are interleaved into the V
    bounce rows on the SBUF side, so they ride through the collective and
    the unpack DMA is fully contiguous.
  - LN1/LN2 means and rstd come from bn_stats; the normalize-scale-shift is
    a single fused scalar-engine activation.  nx is transposed feature-major
    with PE transposes (no DRAM round trip).
  - Attention is exp(ACT)-paced: scores run 3 head-pairs ahead of the
    AV accumulation; exp tiles are written to fp8 and consumed by
    DoubleRow AV matmuls (contraction pairs of key chunks); junk-matmul
    "flip bursts" keep the PE HAM clock-gate at full rate through the
    ACT-paced stretches.
  - The FFN uses constant-scale activation quantization: hq = fp8(LN2*32)
    with the dequant s1/32 folded into the gelu activation scale, and the
    gelu output is written straight to fp8 (its rounding replaces the
    reference's per-token int8 grid; dequant s2 is constant).  mm1 is
    FF-major (lhsT = w1 pairs), mm2 token-major (lhsT = y1q pairs), both
    fp8 DoubleRow, so no activation matrix is ever transposed through DRAM;
    hq needs 32 PE-transposes.
"""

import numpy as np
import ml_dtypes

import concourse.bacc as bacc
import concourse.bass as bass
import concourse.mybir as mybir
import concourse.tile as tile
from concourse.bass_interp import get_hw_module
from concourse.bass_utils import run_bass_kernel_spmd
from concourse.masks import make_identity

F32 = mybir.dt.float32
BF16 = mybir.dt.bfloat16
FP8 = mybir.dt.float8e4
AF = mybir.ActivationFunctionType
OP = mybir.AluOpType
DR = mybir.MatmulPerfMode.DoubleRow
DRSW = mybir.MatmulPerfMode.DoubleRowSwInterleave

N_CORES = 8
B, S, D, H, FF = 2, 2048, 1024, 16, 4096
HD = D // H                 # 64
NTOK = B * S                # 4096
TOK = NTOK // N_CORES       # 512 tokens per core
TCH = TOK // 128            # 4 token chunks per core
DCH = D // 128              # 8
FFCH = FF // 128            # 32
NKC = S // 128              # 16 key chunks per batch
GROUPS = [[0, 1, 2, 3], [4, 5, 6, 7]]
CORES_PER_B = 4
EPS = 1e-5
MAGIC = 12582912.0          # 1.5 * 2**23: fp32 round-to-nearest-even trick
INV_SQRT_HD = 1.0 / 8.0
GELU_MIN = 0.17             # |min gelu| = 0.16995 (reference for error analysis)
QC = 32.0                   # constant act-quant scale: hq = fp8(LN2(x2)*QC)
SCH_A = 12102203.16 / 8.0   # Schraudolph exp(s/8): 2^23/ln2 scaled by 1/8
SCH_B = float((127 << 23) - 368000)


def _bcast_part(ap, parts):
    """View a [1, F] (or [F]) AP as [parts, F] via a zero-stride partition dim."""
    inner = [list(e) for e in ap.ap if e[1] != 1] or [[1, 1]]
    return bass.AP(tensor=ap.tensor, offset=ap.offset, ap=[[0, parts]] + inner)


def build_program(s1, s2, biases, sim_gelu=False):
    nc = bacc.Bacc("TRN2", target_bir_lowering=False, debug=False,
                   num_devices=N_CORES)

    x_in = nc.dram_tensor("x_sh", [TOK, D], BF16, kind="ExternalInput")
    wq_in = nc.dram_tensor("wqT", [D, D], BF16, kind="ExternalInput")
    wk_in = nc.dram_tensor("wkT", [D, D], BF16, kind="ExternalInput")
    wv_in = nc.dram_tensor("wvT", [D, D], BF16, kind="ExternalInput")
    wo_in = nc.dram_tensor("woT", [D, D], BF16, kind="ExternalInput")
    w1_in = nc.dram_tensor("w1T", [D, FF], FP8, kind="ExternalInput")
    w2_in = nc.dram_tensor("w2T", [128, (FFCH // 2) * D * 2], FP8, kind="ExternalInput")
    ident_in = nc.dram_tensor("ident", [128, 128], BF16, kind="ExternalInput")
    out_d = nc.dram_tensor("out", [TOK, D], F32, kind="ExternalOutput")

    ext = {}
    if biases["ln1_g"]:
        ext["ln1_g"] = nc.dram_tensor("ln1_g", [D], F32, kind="ExternalInput")
    if biases["ln1_b"]:
        ext["ln1_b"] = nc.dram_tensor("ln1_b", [D], F32, kind="ExternalInput")
    if biases["ln2_g"]:
        ext["ln2_g"] = nc.dram_tensor("ln2_g", [D], F32, kind="ExternalInput")
    if biases["ln2_b"]:
        ext["ln2_b"] = nc.dram_tensor("ln2_b", [D], F32, kind="ExternalInput")
    if biases["in_proj_b"]:
        ext["in_b"] = nc.dram_tensor("in_b", [3 * D], F32, kind="ExternalInput")
    if biases["out_proj_b"]:
        ext["out_b"] = nc.dram_tensor("out_b", [D], F32, kind="ExternalInput")
    if biases["b1"]:
        ext["b1"] = nc.dram_tensor("b1", [FF], F32, kind="ExternalInput")
    if biases["b2"]:
        ext["b2"] = nc.dram_tensor("b2", [D], F32, kind="ExternalInput")

    with tile.TileContext(nc) as tc:
        _emit(nc, tc, x_in, wq_in, wk_in, wv_in, wo_in, w1_in, w2_in,
              ident_in, out_d, ext, s1, s2, biases, sim_gelu)
    nc.compile()
    return nc


def _scope(nc, name):
    sid = nc.enter_named_scope(name, False)
    return (name, sid[0] if isinstance(sid, tuple) else sid)


def _unscope(nc, tok):
    nc.leave_named_scope(tok[0], tok[1], False)


def _emit(nc, tc, x_in, wq_in, wk_in, wv_in, wo_in, w1_in, w2_in, ident_in,
          out_d, ext, s1, s2, biases, sim_gelu=False):
    gelu_func = AF.Tanh if sim_gelu else AF.Gelu
    from contextlib import ExitStack

    es_top = ExitStack()
    dram = es_top.enter_context(tc.tile_pool(name="dram", bufs=1, space="DRAM"))
    const = es_top.enter_context(tc.tile_pool(name="const", bufs=1))
    stats = es_top.enter_context(tc.tile_pool(name="stats", bufs=4))

    VA = H * (HD + 1)           # 1040: V row with a ones column per head
    kT_bounce = dram.tile([D, TOK], FP8)
    v_bounce1 = dram.tile([TOK, VA // 2], FP8)
    v_bounce2 = dram.tile([TOK, VA // 2], FP8)
    kT_all1 = dram.tile([CORES_PER_B * (D // 2), TOK], FP8)
    kT_all2 = dram.tile([CORES_PER_B * (D // 2), TOK], FP8)
    v_all1 = dram.tile([S, VA // 2], FP8)
    v_all2 = dram.tile([S, VA // 2], FP8)
    den_dram = dram.tile([H, TOK], F32)
    dq1_dram = dram.tile([TOK], F32)
    a2_dram = dram.tile([TOK], F32)

    eps_t = const.tile([128, 1], F32)
    nc.vector.memset(eps_t[:], EPS)
    magic_t = const.tile([128, 1], F32)
    nc.vector.memset(magic_t[:], MAGIC)
    ident = const.tile([128, 128], BF16, tag="ident")
    nc.sync.dma_start(out=ident[:], in_=ident_in[:])

    def load_bcast(name, width, src_ap):
        t = const.tile([128, width], F32, tag=f"bc_{name}")
        nc.sync.dma_start(out=t[:], in_=_bcast_part(src_ap, 128))
        return t

    g1_bc = load_bcast("g1", D, ext["ln1_g"][:]) if biases["ln1_g"] else None
    b1ln_bc = load_bcast("b1ln", D, ext["ln1_b"][:]) if biases["ln1_b"] else None
    g2_bc = load_bcast("g2", D, ext["ln2_g"][:]) if biases["ln2_g"] else None
    b2ln_bc = load_bcast("b2ln", D, ext["ln2_b"][:]) if biases["ln2_b"] else None
    bv_bc = (load_bcast("bv", D, ext["in_b"][2 * D:3 * D])
             if biases["in_proj_b"] else None)
    bo_bc = load_bcast("bo", D, ext["out_b"][:]) if biases["out_proj_b"] else None
    bf2_bc = load_bcast("bf2", D, ext["b2"][:]) if biases["b2"] else None
    if biases["in_proj_b"]:
        bq_fm = const.tile([128, DCH], F32, tag="bq_fm")
        nc.sync.dma_start(out=bq_fm[:], in_=ext["in_b"][0:D].rearrange("(c p) -> p c", p=128))
        bk_fm = const.tile([128, DCH], F32, tag="bk_fm")
        nc.sync.dma_start(out=bk_fm[:], in_=ext["in_b"][D:2 * D].rearrange("(c p) -> p c", p=128))
    if biases["b1"]:
        b1_fm = const.tile([128, FFCH], F32, tag="b1_fm")
        nc.sync.dma_start(out=b1_fm[:], in_=ext["b1"][:].rearrange("(c p) -> p c", p=128))

    # ---- long-lived pools (stack allocator: open early, close late) -----
    es_D = ExitStack()
    pD = es_D.enter_context(tc.tile_pool(name="pD", bufs=1))
    x2 = pD.tile([128, TCH, D], F32, tag="x2")
    s2t = pD.tile([128, 1], F32, tag="s2t")
    nc.vector.memset(s2t[:], float(s2))

    es_W1 = ExitStack()
    pW1 = es_W1.enter_context(tc.tile_pool(name="pW1", bufs=1))
    w1_sb = pW1.tile([128, DCH, FF], FP8, tag="w1")  # 32KB/part

    es_A = ExitStack()
    pA = es_A.enter_context(tc.tile_pool(name="pA", bufs=1))
    x_bf = pA.tile([128, TCH, D], BF16, tag="x")

    es_C = ExitStack()
    pC = es_C.enter_context(tc.tile_pool(name="pC", bufs=1))
    oT = pC.tile([128, DCH, TOK], BF16, tag="oT")

    es_B = ExitStack()
    pB = es_B.enter_context(tc.tile_pool(name="pB", bufs=1))
    qT = pB.tile([128, DCH, TOK], BF16, tag="qT")
    KT = pB.tile([128, DCH, CORES_PER_B, 512], FP8, tag="KT")
    Vaug = pB.tile([128, NKC, H * (HD + 1)], FP8, tag="Va")

    # ---- stage 0: load x (bf16) -----------------------------------------
    x_v = x_in.rearrange("(i p) d -> p i d", p=128)
    for i in range(TCH):
        nc.sync.dma_start(out=x_bf[:, i, :], in_=x_v[:, i, :])

    # PE warm-up: keep the HAM clock-gate at 8/8 through the prologue so the
    # first real matmuls run at full clock.  Junk matmuls on the identity.
    tok_wm = _scope(nc, "warm")
    es_wm = ExitStack()
    ps_wm = es_wm.enter_context(tc.tile_pool(name="ps_wm", bufs=1, space="PSUM"))
    wm = ps_wm.tile([128, 128], F32, tag="wm")
    for _ in range(64):
        nc.tensor.matmul(wm[:], lhsT=ident[:], rhs=ident[:], start=True, stop=True)
    es_wm.close()
    _unscope(nc, tok_wm)

    # ---- stage 1: LN1 -> nx (bf16, token-major) -> DRAM -----------------
    def layer_norm_chunk(src_ap, g_bc, b_bc, out_tile, post_scale=1.0):
        """out = (src - m) * rstd * post_scale [* g] [+ b] via one fused ACT op."""
        st = stats.tile([128, 2, 6], F32, tag="bnst")
        nc.vector.bn_stats(out=st[:, 0, :], in_=src_ap[:, 0:512])
        nc.vector.bn_stats(out=st[:, 1, :], in_=src_ap[:, 512:1024])
        mv = stats.tile([128, 2], F32, tag="mv")
        nc.vector.bn_aggr(out=mv[:], in_=st[:])
        r = stats.tile([128, 1], F32, tag="rstd")
        nc.scalar.activation(out=r[:], in_=mv[:, 1:2], func=AF.Ln, bias=eps_t[:])
        nc.scalar.activation(out=r[:], in_=r[:], func=AF.Exp, scale=-0.5)
        fold = post_scale != 1.0 and g_bc is None and b_bc is None
        if fold:
            nc.vector.tensor_scalar_mul(out=r[:], in0=r[:], scalar1=float(post_scale))
        nmr = stats.tile([128, 1], F32, tag="nmr")
        nc.vector.tensor_tensor(out=nmr[:], in0=mv[:, 0:1], in1=r[:], op=OP.mult)
        nc.vector.tensor_scalar_mul(out=nmr[:], in0=nmr[:], scalar1=-1.0)
        nc.scalar.activation(out=out_tile, in_=src_ap, func=AF.Identity,
                             scale=r[:], bias=nmr[:])
        if g_bc is not None:
            nc.vector.tensor_mul(out=out_tile, in0=out_tile, in1=g_bc[:])
        if b_bc is not None:
            nc.vector.tensor_add(out=out_tile, in0=out_tile, in1=b_bc[:])
        if post_scale != 1.0 and not fold:
            nc.vector.tensor_scalar_mul(out=out_tile, in0=out_tile,
                                        scalar1=float(post_scale))
        return mv, r

    # weight loads first so they don't queue behind LN1-dependent DMAs
    es_X = ExitStack()
    pX = es_X.enter_context(tc.tile_pool(name="pX", bufs=1))
    pW = es_X.enter_context(tc.tile_pool(name="pWqkv", bufs=1))
    wk_sb = pW.tile([128, DCH, D], BF16, tag="wk")
    nc.sync.dma_start(out=wk_sb[:], in_=wk_in.rearrange("(c p) f -> p c f", p=128))
    wv_sb = pW.tile([128, DCH, D], BF16, tag="wv")
    nc.sync.dma_start(out=wv_sb[:], in_=wv_in.rearrange("(c p) f -> p c f", p=128))
    wq_sb = pW.tile([128, DCH, D], BF16, tag="wq")
    nc.sync.dma_start(out=wq_sb[:], in_=wq_in.rearrange("(c p) f -> p c f", p=128))

    tok_ln1 = _scope(nc, "ln1")
    nxT = pX.tile([128, DCH, TOK], BF16, tag="nxT")
    es_1 = ExitStack()
    s1p = es_1.enter_context(tc.tile_pool(name="s1p", bufs=3))
    ps1 = es_1.enter_context(tc.tile_pool(name="ps1", bufs=4, space="PSUM"))
    for i in range(TCH):
        nxt = s1p.tile([128, D], BF16, tag="nx")
        layer_norm_chunk(x_bf[:, i, :], g1_bc, b1ln_bc, nxt[:])
        for dc in range(DCH):
            pst = ps1.tile([128, 128], BF16, tag="ps1t")
            nc.tensor.transpose(pst[:], nxt[:, dc * 128:(dc + 1) * 128], ident[:])
            nc.vector.tensor_copy(out=nxT[:, dc, i * 128:(i + 1) * 128], in_=pst[:])
    es_1.close()
    _unscope(nc, tok_ln1)

    tok_inp = _scope(nc, "inproj")
    es_3 = ExitStack()
    ps3 = es_3.enter_context(tc.tile_pool(name="ps3", bufs=4, space="PSUM"))
    s3 = es_3.enter_context(tc.tile_pool(name="s3", bufs=3))

    # k projection, feature-major, cast fp8 -> bounce; the K AllGather is
    # split in two so the first half (heads 0-7) lands much earlier
    def k_half(half, chunked=False):
        for fo in range(half * 4, half * 4 + 4):
            ps = ps3.tile([128, 512], F32, tag="ps")
            if chunked:
                # token-chunked so the first chunks only need the first LN1
                # outputs -> the K bounce (and its AllGather) starts earliest
                for tch in range(TCH):
                    sl = slice(tch * 128, (tch + 1) * 128)
                    for dc in range(DCH):
                        nc.tensor.matmul(ps[:, sl],
                                         lhsT=wk_sb[:, dc, fo * 128:(fo + 1) * 128],
                                         rhs=nxT[:, dc, sl],
                                         start=(dc == 0), stop=(dc == DCH - 1))
            else:
                for dc in range(DCH):
                    nc.tensor.matmul(ps[:], lhsT=wk_sb[:, dc, fo * 128:(fo + 1) * 128],
                                     rhs=nxT[:, dc, :], start=(dc == 0),
                                     stop=(dc == DCH - 1))
            kc_sb = s3.tile([128, 512], FP8, tag="kcp")
            if biases["in_proj_b"]:
                nc.scalar.activation(out=kc_sb[:], in_=ps[:], func=AF.Identity,
                                     bias=bk_fm[:, fo:fo + 1])
            else:
                nc.vector.tensor_copy(out=kc_sb[:], in_=ps[:])
            nc.sync.dma_start(
                out=kT_bounce.rearrange("(c p) t -> p c t", p=128)[:, fo, :],
                in_=kc_sb[:])

    k_half(0, chunked=True)
    tok_cK = _scope(nc, "collK")
    nc.gpsimd.collective_compute(
        "AllGather", OP.bypass, replica_groups=GROUPS,
        ins=[kT_bounce[0:D // 2, :].opt()], outs=[kT_all1.opt()])
    _unscope(nc, tok_cK)
    _unscope(nc, tok_inp)

    tok_vp = _scope(nc, "vproj")
    # v projection, token-major, cast fp8, interleave per-head ones columns
    # (the ones ride through the collective so the unpack DMA is contiguous)
    for to in range(TCH):
        vci = s3.tile([128, H, HD + 1], FP8, tag="vci", name=f"vci{to}")
        nc.vector.memset(vci[:], 1.0)
        pv = [ps3.tile([128, 512], F32, tag="ps", name=f"psv{to}_{i}")
              for i in range(2)]
        for dc in range(DCH):
            for f2 in range(2):
                nc.tensor.matmul(pv[f2][:], lhsT=nxT[:, dc, to * 128:(to + 1) * 128],
                                 rhs=wv_sb[:, dc, f2 * 512:(f2 + 1) * 512],
                                 start=(dc == 0), stop=(dc == DCH - 1))
        for f2 in range(2):
            dst = vci[:, f2 * 8:(f2 + 1) * 8, 0:HD]
            if biases["in_proj_b"]:
                nc.vector.tensor_add(out=dst, in0=pv[f2][:],
                                     in1=bv_bc[:, f2 * 512:(f2 + 1) * 512])
            else:
                nc.vector.tensor_copy(out=dst, in_=pv[f2][:])
        nc.sync.dma_start(
            out=v_bounce1.rearrange("(i p) f -> p i f", p=128)[:, to, :],
            in_=vci[:, 0:8, :])
        nc.sync.dma_start(
            out=v_bounce2.rearrange("(i p) f -> p i f", p=128)[:, to, :],
            in_=vci[:, 8:16, :])
    _unscope(nc, tok_vp)

    tok_cV = _scope(nc, "collV")
    nc.gpsimd.collective_compute(
        "AllGather", OP.bypass, replica_groups=GROUPS,
        ins=[v_bounce1.opt()], outs=[v_all1.opt()])
    _unscope(nc, tok_cV)

    tok_qp = _scope(nc, "qproj")
    for fo in range(DCH):
        ps = ps3.tile([128, 512], F32, tag="ps")
        for dc in range(DCH):
            nc.tensor.matmul(ps[:], lhsT=wq_sb[:, dc, fo * 128:(fo + 1) * 128],
                             rhs=nxT[:, dc, :], start=(dc == 0), stop=(dc == DCH - 1))
        if biases["in_proj_b"]:
            nc.scalar.activation(out=qT[:, fo, :], in_=ps[:], func=AF.Identity,
                                 bias=bq_fm[:, fo:fo + 1])
        else:
            nc.scalar.activation(out=qT[:, fo, :], in_=ps[:], func=AF.Copy)
    _unscope(nc, tok_qp)

    tok_k2 = _scope(nc, "kproj2")
    k_half(1)
    _unscope(nc, tok_k2)
    tok_cK2 = _scope(nc, "collK2")
    nc.gpsimd.collective_compute(
        "AllGather", OP.bypass, replica_groups=GROUPS,
        ins=[kT_bounce[D // 2:D, :].opt()], outs=[kT_all2.opt()])
    _unscope(nc, tok_cK2)
    tok_cV2 = _scope(nc, "collV2")
    nc.gpsimd.collective_compute(
        "AllGather", OP.bypass, replica_groups=GROUPS,
        ins=[v_bounce2.opt()], outs=[v_all2.opt()])
    _unscope(nc, tok_cV2)

    # prefetch FFN + out_proj weights (overlaps collectives/attention)
    nc.sync.dma_start(out=w1_sb[:], in_=w1_in.rearrange("(c p) f -> p c f", p=128))
    wo_sb = pC.tile([128, DCH, D], BF16, tag="wo")
    nc.sync.dma_start(out=wo_sb[:], in_=wo_in.rearrange("(c p) f -> p c f", p=128))

    tok_un = _scope(nc, "unpack")
    # unpack gathered K^T / V(+ones) into SBUF (fp8, all contiguous runs)
    HD2 = D // 2
    for c in range(CORES_PER_B):
        nc.sync.dma_start(
            out=KT[:, 0:4, c, :],
            in_=kT_all1[c * HD2:(c + 1) * HD2, :].rearrange("(dch p) t -> p dch t", p=128))
    for c in range(CORES_PER_B):
        nc.sync.dma_start(
            out=KT[:, 4:8, c, :],
            in_=kT_all2[c * HD2:(c + 1) * HD2, :].rearrange("(dch p) t -> p dch t", p=128))
    va1 = v_all1.rearrange("(kc p) f -> p kc f", p=128)
    nc.sync.dma_start(out=Vaug[:, :, 0:VA // 2], in_=va1[:])
    va2 = v_all2.rearrange("(kc p) f -> p kc f", p=128)
    nc.sync.dma_start(out=Vaug[:, :, VA // 2:VA], in_=va2[:])
    es_3.close()
    es_X.close()
    _unscope(nc, tok_un)

    tok_at = _scope(nc, "attn")
    # ---- attention: ACT(exp)-paced pipeline, AV lags scores by 3 hp ------
    es_5 = ExitStack()
    ps_s = es_5.enter_context(tc.tile_pool(name="ps_s", bufs=3, space="PSUM"))
    ps_av = es_5.enter_context(tc.tile_pool(name="ps_av", bufs=2, space="PSUM"))
    s5e = es_5.enter_context(tc.tile_pool(name="s5e", bufs=60))
    s5d = es_5.enter_context(tc.tile_pool(name="s5d", bufs=3))

    exp_tiles = {}

    def emit_scores(hp):
        for g in range(NKC // 2):
            pss = [ps_s.tile([128, 2, 512], F32, tag="pss", name=f"pss{hp}_{g}_{i}")
                   for i in range(2)]
            for j in range(2):
                kc = 2 * g + j
                c, tcc = divmod(kc, 4)
                ksl = KT[:, hp, c, tcc * 128:(tcc + 1) * 128]
                nc.tensor.matmul(pss[0][:, j, :], lhsT=ksl[0:64, :],
                                 rhs=qT[0:64, hp, :], start=True, stop=True,
                                 tile_position=(0, 0))
                nc.tensor.matmul(pss[1][:, j, :], lhsT=ksl[64:128, :],
                                 rhs=qT[64:128, hp, :], start=True, stop=True,
                                 tile_position=(64, 0))
            for jh in range(2):
                e = s5e.tile([128, 512, 2], FP8, tag="exp", name=f"e{hp}_{g}_{jh}")
                nc.scalar.activation(out=e[:].rearrange("p t j -> p j t"),
                                     in_=pss[jh][:], func=AF.Exp,
                                     scale=INV_SQRT_HD)
                exp_tiles[(hp, jh, g)] = e

    def emit_av(hp):
        for jh in range(2):
            h = 2 * hp + jh
            pav = ps_av.tile([128, 512], F32, tag="pav", name=f"pav{hp}_{jh}")
            for g in range(NKC // 2):
                nc.tensor.matmul(pav[0:65, :],
                                 lhsT=Vaug[:, 2 * g:2 * g + 2, h * 65:h * 65 + 65],
                                 rhs=exp_tiles.pop((hp, jh, g))[:].rearrange("p t j -> p j t"),
                                 start=(g == 0), stop=(g == NKC // 2 - 1),
                                 perf_mode=DR)
            oun = s5d.tile([65, 512], F32, tag="oun", name=f"oun{hp}_{jh}")
            nc.vector.tensor_copy(out=oun[:], in_=pav[0:65, :])
            nc.sync.dma_start(out=den_dram[h:h + 1, :], in_=oun[64:65, :])
            dbc = s5d.tile([64, 512], F32, tag="dbc", name=f"dbc{hp}_{jh}")
            nc.sync.dma_start(out=dbc[:], in_=_bcast_part(den_dram[h:h + 1, :], 64))
            rdb = s5d.tile([64, 512], F32, tag="rdb", name=f"rdb{hp}_{jh}")
            nc.vector.reciprocal(out=rdb[:], in_=dbc[:])
            nc.vector.tensor_mul(out=oT[jh * 64:jh * 64 + 64, hp, :],
                                 in0=oun[0:64, :], in1=rdb[:])

    def emit_flip(hp):
        # one contiguous PE burst to flip the HAM clock-gate back to 8/8 if a
        # stall mid-attention dropped it to 4/8
        jp = ps_av.tile([128, 512], F32, tag="pav", name=f"flip{hp}")
        for i in range(44):
            nc.tensor.matmul(jp[:, 0:128], lhsT=ident[:], rhs=ident[:],
                             start=True, stop=True, skip_group_check=True)

    PIPE = 3
    for hp in range(PIPE):
        emit_scores(hp)
    for hp in range(H // 2):
        emit_av(hp)
        if hp < H // 2 - 1:
            emit_flip(hp)
        if hp + PIPE < H // 2:
            emit_scores(hp + PIPE)
    es_5.close()
    es_B.close()
    _unscope(nc, tok_at)

    tok_op = _scope(nc, "outproj")
    # ---- out_proj + residual (token-major) ------------------------------
    es_6 = ExitStack()
    ps6 = es_6.enter_context(tc.tile_pool(name="ps6", bufs=4, space="PSUM"))
    for to in range(TCH):
        po = [ps6.tile([128, 512], F32, tag="ps6", name=f"pso{to}_{i}")
              for i in range(2)]
        for dc in range(DCH):
            for f2 in range(2):
                nc.tensor.matmul(po[f2][:], lhsT=oT[:, dc, to * 128:(to + 1) * 128],
                                 rhs=wo_sb[:, dc, f2 * 512:(f2 + 1) * 512],
                                 start=(dc == 0), stop=(dc == DCH - 1))
        for f2 in range(2):
            dst = x2[:, to, f2 * 512:(f2 + 1) * 512]
            nc.vector.tensor_add(out=dst, in0=po[f2][:],
                                 in1=x_bf[:, to, f2 * 512:(f2 + 1) * 512])
            if biases["out_proj_b"]:
                nc.vector.tensor_add(out=dst, in0=dst,
                                     in1=bo_bc[:, f2 * 512:(f2 + 1) * 512])
    es_6.close()
    es_C.close()
    es_A.close()
    _unscope(nc, tok_op)

    tok_l2 = _scope(nc, "ln2q")
    # ---- LN2 + constant-scale act_quant -> hqT (fp8, feature-major) -----
    # hq = fp8(clamp(LN2(x2) * QC)); the dequant (s1/QC) folds into the gelu
    # scale, so no per-token amax/scale machinery is needed.
    es_E = ExitStack()
    pE = es_E.enter_context(tc.tile_pool(name="pE", bufs=1))
    hqT = pE.tile([128, DCH // 2, TOK, 2], FP8, tag="hqT")
    y1qT = pE.tile([128, FFCH, TOK], FP8, tag="y1qT")

    es_7 = ExitStack()
    s7 = es_7.enter_context(tc.tile_pool(name="s7", bufs=3))
    ps7 = es_7.enter_context(tc.tile_pool(name="ps7", bufs=4, space="PSUM"))

    for to in range(TCH):
        hs = s7.tile([128, D], BF16, tag="hs")
        layer_norm_chunk(x2[:, to, :], g2_bc, b2ln_bc, hs[:], post_scale=QC)
        hqi = s7.tile([128, D], BF16, tag="hq")
        nc.vector.tensor_scalar(out=hqi[:], in0=hs[:], scalar1=239.0,
                                scalar2=-239.0, op0=OP.min, op1=OP.max)
        for dc in range(DCH):
            pst = ps7.tile([128, 128], BF16, tag="pst")
            nc.tensor.transpose(pst[:], hqi[:, dc * 128:(dc + 1) * 128], ident[:])
            nc.vector.tensor_copy(
                out=hqT[:, dc // 2, to * 128:(to + 1) * 128, dc % 2],
                in_=pst[:])
    es_7.close()
    _unscope(nc, tok_l2)

    tok_f1 = _scope(nc, "ffn1")
    # ---- FFN mm1 (fp8 DoubleRow, FF-major) + gelu + running max ----------
    es_8 = ExitStack()
    ps8 = es_8.enter_context(tc.tile_pool(name="ps8", bufs=4, space="PSUM"))
    s8 = es_8.enter_context(tc.tile_pool(name="s8", bufs=3))
    es_W2 = ExitStack()
    pW2 = es_W2.enter_context(tc.tile_pool(name="pW2", bufs=1))
    w2_sb = pW2.tile([128, FFCH // 2, D, 2], FP8, tag="w2")
    nc.sync.dma_start(out=w2_sb[:], in_=w2_in[:])

    for ffo in range(FFCH):
        ps = ps8.tile([128, 512], F32, tag="ps8")
        for dcp in range(DCH // 2):
            nc.tensor.matmul(ps[:],
                             lhsT=w1_sb[:, 2 * dcp:2 * dcp + 2, ffo * 128:(ffo + 1) * 128],
                             rhs=hqT[:, dcp, :, :].rearrange("p t j -> p j t"),
                             start=(dcp == 0), stop=(dcp == DCH // 2 - 1),
                             perf_mode=DR)
        # y1 = ps * (s1/QC); gelu output goes straight to fp8 (its rounding
        # replaces the reference's int8 act_quant grid)
        if biases["b1"]:
            nc.scalar.activation(out=y1qT[:, ffo, :], in_=ps[:], func=gelu_func,
                                 scale=float(s1) / QC, bias=b1_fm[:, ffo:ffo + 1])
        else:
            nc.scalar.activation(out=y1qT[:, ffo, :], in_=ps[:], func=gelu_func,
                                 scale=float(s1) / QC)
    _unscope(nc, tok_f1)

    tok_f2 = _scope(nc, "ffn2")
    # ---- FFN mm2 (fp8 DoubleRow, token-major) + residual -> out ----------
    out_v = out_d.rearrange("(i p) d -> p i d", p=128)
    es_9 = ExitStack()
    ps9 = es_9.enter_context(tc.tile_pool(name="ps9", bufs=4, space="PSUM"))
    s9 = es_9.enter_context(tc.tile_pool(name="s9", bufs=3))
    for to in range(TCH):
        p9 = [ps9.tile([128, 512], F32, tag="ps9", name=f"ps9_{to}_{i}")
              for i in range(2)]
        for fcp in range(FFCH // 2):
            for f2 in range(2):
                nc.tensor.matmul(p9[f2][:],
                                 lhsT=y1qT[:, 2 * fcp:2 * fcp + 2, to * 128:(to + 1) * 128],
                                 rhs=w2_sb[:, fcp, f2 * 512:(f2 + 1) * 512, :].rearrange("p t j -> p j t"),
                                 start=(fcp == 0), stop=(fcp == FFCH // 2 - 1),
                                 perf_mode=DR)
        for f2 in range(2):
            outt = s9.tile([128, 512], F32, tag="outt")
            nc.vector.scalar_tensor_tensor(
                out=outt[:], in0=p9[f2][:], scalar=s2t[:],
                in1=x2[:, to, f2 * 512:(f2 + 1) * 512], op0=OP.mult, op1=OP.add)
            if biases["b2"]:
                nc.vector.tensor_add(out=outt[:], in0=outt[:],
                                     in1=bf2_bc[:, f2 * 512:(f2 + 1) * 512])
            nc.sync.dma_start(out=out_v[:, to, f2 * 512:(f2 + 1) * 512],
                              in_=outt[:])
    _unscope(nc, tok_f2)
    es_9.close()
    es_W2.close()
    es_8.close()
    es_E.close()
    es_W1.close()
    es_D.close()
    es_top.close()


_CACHE = {}


def _prepare(inputs):
    bf = ml_dtypes.bfloat16
    f8 = ml_dtypes.float8_e4m3
    x = np.ascontiguousarray(np.asarray(inputs["x"], dtype=np.float32))
    in_w = np.asarray(inputs["in_proj_w"], dtype=np.float32)
    out_w = np.asarray(inputs["out_proj_w"], dtype=np.float32)
    w1 = np.asarray(inputs["w1"], dtype=np.float32)
    w2 = np.asarray(inputs["w2"], dtype=np.float32)

    s1 = float(max(np.mean(np.abs(w1), dtype=np.float32), EPS))
    s2 = float(max(np.mean(np.abs(w2), dtype=np.float32), EPS))
    t1 = np.clip(np.round(w1 / np.float32(s1)), -1.0, 1.0).astype(np.float32)
    t2 = np.clip(np.round(w2 / np.float32(s2)), -1.0, 1.0).astype(np.float32)

    host = {
        "wqT": np.ascontiguousarray(in_w[0:D].T).astype(bf),
        "wkT": np.ascontiguousarray(in_w[D:2 * D].T).astype(bf),
        "wvT": np.ascontiguousarray(in_w[2 * D:3 * D].T).astype(bf),
        "woT": np.ascontiguousarray(out_w.T).astype(bf),
        "w1T": np.ascontiguousarray(t1.T).astype(f8),
        # w2 moving operand, pair-interleaved: [128, FFCH//2, D, 2]
        "w2T": np.ascontiguousarray(
            t2.T.reshape(16, 2, 128, D).transpose(2, 0, 3, 1)
            .reshape(128, -1)).astype(f8),
        "ident": np.eye(128, dtype=bf),
    }

    def nz(a):
        return bool(np.any(np.asarray(a) != 0.0))

    biases = {
        "ln1_g": bool(np.any(np.asarray(inputs["ln1_g"]) != 1.0)),
        "ln1_b": nz(inputs["ln1_b"]),
        "ln2_g": bool(np.any(np.asarray(inputs["ln2_g"]) != 1.0)),
        "ln2_b": nz(inputs["ln2_b"]),
        "in_proj_b": nz(inputs["in_proj_b"]),
        "out_proj_b": nz(inputs["out_proj_b"]),
        "b1": nz(inputs["b1"]),
        "b2": nz(inputs["b2"]),
    }
    extra = {}
    if biases["ln1_g"]:
        extra["ln1_g"] = np.asarray(inputs["ln1_g"], np.float32)
    if biases["ln1_b"]:
        extra["ln1_b"] = np.asarray(inputs["ln1_b"], np.float32)
    if biases["ln2_g"]:
        extra["ln2_g"] = np.asarray(inputs["ln2_g"], np.float32)
    if biases["ln2_b"]:
        extra["ln2_b"] = np.asarray(inputs["ln2_b"], np.float32)
    if biases["in_proj_b"]:
        extra["in_b"] = np.asarray(inputs["in_proj_b"], np.float32)
    if biases["out_proj_b"]:
        extra["out_b"] = np.asarray(inputs["out_proj_b"], np.float32)
    if biases["b1"]:
        extra["b1"] = np.asarray(inputs["b1"], np.float32)
    if biases["b2"]:
        extra["b2"] = np.asarray(inputs["b2"], np.float32)

    x_flat = x.reshape(NTOK, D).astype(bf)
    in_maps = []
    for c in range(N_CORES):
        m = {"x_sh": np.ascontiguousarray(x_flat[c * TOK:(c + 1) * TOK])}
        m.update(host)
        m.update(extra)
        in_maps.append(m)
    return in_maps, s1, s2, biases


def get_program(s1, s2, biases, for_hw=True, sim_gelu=False):
    key = (round(s1, 12), round(s2, 12), tuple(sorted(biases.items())), for_hw,
           sim_gelu)
    if key not in _CACHE:
        nc = build_program(s1, s2, biases, sim_gelu=sim_gelu)
        if for_hw:
            nc.m = get_hw_module(nc.m)
        _CACHE[key] = nc
    return _CACHE[key]


def kernel(**inputs):
    in_maps, s1, s2, biases = _prepare(inputs)
    nc = get_program(s1, s2, biases, for_hw=True)
    res = run_bass_kernel_spmd(nc, in_maps, list(range(N_CORES)))
    out = np.concatenate([res.results[c]["out"] for c in range(N_CORES)], axis=0)
    return out.reshape(B, S, D).astype(np.float32)
